# revision 17
# baseline (speedup 1.0000x reference)
"""Biaffine edge attention on 8 Trainium2 NeuronCores.

out[b,i,j] = head[b,i,:] @ U @ dep[b,j,:] + head[b,i,:]@w1 + dep[b,j,:]@w2 + b0

Sharding: data-parallel over batch (B=8, one batch per core).

Formulation (all relayout / algebraic folding done host-side):
  HT = head[b].T                               (host relayout)
  T1T[k,i] = sum_d U[d,k] * HT[d,i]            (mm1, device)
  t1t[k,i] = T1T[k,i] + w2[k]                  (fold into the PSUM->SBUF
                                                copy: yields +s_dep[j] after
                                                mm2, since s_dep[j] =
                                                sum_k w2[k] PT[k,j])
  PT'[k,j] = dep[b].T[k,j] + c[k],  U c = w1   (host solve + relayout:
                                                yields +s_head[i] after mm2,
                                                since sum_k T1T[k,i] c[k] =
                                                head_i @ (U c) = s_head[i])
  out[i,j] = sum_k t1t[k,i] * PT'[k,j] + b0'   b0' = b0 - w2 . c

Device work = the 2 * S*D*D MAC roofline (256 N=512 matmuls) plus 32
bias-add PSUM evacuations (alternating DVE/ACT). No on-device transposes.
Inputs stream as bf16 (rel err ~5e-3 vs the 2e-2 gate), PSUM accumulates
fp32.

Schedule notes (from traces): HWDGE descriptor-gen costs ~0.7us per
dma_start regardless of size, per-ring transfers are FIFO, and the two
HWDGE rings (sync + scalar) share the SDMA engines. Every input chunk is
its own fully-contiguous dram tensor (dense HBM bursts); the two chunks
needed first (u kt=0 and ht h0/do0-3) gen in parallel on the two rings.
ht h1 descriptors are generated mid-stream from the ACT queue so the 1MB
transfer does not compete with the critical startup chunks. A burst of
matmuls on a zeroed tile keeps the PE clock warm (HAM) until the first
real data lands; mm2 merges each row-block into one [P,1024] out DMA.
"""

import numpy as np
import ml_dtypes

import concourse.bass as bass
import concourse.mybir as mybir
import concourse.tile as tile
from concourse import bacc
from concourse.bass_utils import run_bass_kernel_spmd

B, S, D = 8, 1024, 1024
P = 128
DO = D // P   # 8
SO = S // P   # 8
NH = 512      # matmul free-dim tile (one fp32 PSUM bank)
F32 = mybir.dt.float32
BF16 = mybir.dt.bfloat16
ADD = mybir.AluOpType.add
BF = ml_dtypes.bfloat16

N_DUMMY = 12  # HAM warm-up matmuls on zeroed data during initial DMA fill

_CACHE = {}


def build_nc():
    nc = bacc.Bacc(None, target_bir_lowering=False)

    # one dram tensor per DMA chunk, each contiguous, layout == sbuf tile
    # slice (partition dim first):
    #   ht*[dd, do, i5] = head.T[do*P+dd, i]     (h0 split in two, h1 whole)
    #   pt*[kk, kt, j5] = (dep.T + c)[kt*P+kk, j]
    #   u*[dd, kt, do, k5] = U[do*P+dd, kt*P+k5]
    u0 = nc.dram_tensor("u0", [P, 1, DO, P], BF16, kind="ExternalInput")
    u1 = nc.dram_tensor("u1", [P, 1, DO, P], BF16, kind="ExternalInput")
    u23 = nc.dram_tensor("u23", [P, 2, DO, P], BF16, kind="ExternalInput")
    u47 = nc.dram_tensor("u47", [P, 4, DO, P], BF16, kind="ExternalInput")
    hta = nc.dram_tensor("hta", [P, 4, NH], BF16, kind="ExternalInput")
    htb = nc.dram_tensor("htb", [P, 4, NH], BF16, kind="ExternalInput")
    ht1 = nc.dram_tensor("ht1", [P, DO, NH], BF16, kind="ExternalInput")
    pt0 = nc.dram_tensor("pt0", [P, DO, NH], BF16, kind="ExternalInput")
    pt1 = nc.dram_tensor("pt1", [P, DO, NH], BF16, kind="ExternalInput")
    wb = nc.dram_tensor("wb", [P, 16], F32, kind="ExternalInput")
    out = nc.dram_tensor("out", [S, S], F32, kind="ExternalOutput")

    with tile.TileContext(nc) as tc:
        with (
            tc.tile_pool(name="const", bufs=1) as const,
            tc.tile_pool(name="big", bufs=1) as big,
            tc.tile_pool(name="outp", bufs=3) as outp,
            tc.tile_pool(name="mm_ps", bufs=8, space="PSUM") as mm_ps,
        ):
            wb_sb = const.tile([P, 16], F32)
            dummy = const.tile([P, NH], BF16)

            u_sb = big.tile([P, DO, DO, P], BF16, tag="u")     # [dd, kt, do, k]
            ht_sb = big.tile([P, 2, DO, NH], BF16, tag="ht")   # [dd, h, do, i]
            pt_sb = big.tile([P, 2, DO, NH], BF16, tag="pt")   # [kk, jh, kt, j]
            t1t_sb = big.tile([P, DO, S], BF16, tag="t1t")     # [kk, kt, i]

            # ---- PE warm-up on zeroed data (no DMA dependency) ----------
            nc.gpsimd.memset(dummy[:], 0)
            dps = mm_ps.tile([P, NH], F32, tag="mm")
            for i in range(N_DUMMY):
                nc.tensor.matmul(
                    dps[:], dummy[:, 0:P], dummy[:],
                    start=(i % 5 == 0), stop=(i % 5 == 4 or i == N_DUMMY - 1),
                )

            # ---- input DMAs (gen order = per-ring FIFO priority) --------
            nc.sync.dma_start(u_sb[:, 0:1], u0[:])
            nc.sync.dma_start(u_sb[:, 2:4], u23[:])
            nc.sync.dma_start(u_sb[:, 4:8], u47[:])
            nc.sync.dma_start(pt_sb[:, 0], pt0[:])
            nc.sync.dma_start(pt_sb[:, 1], pt1[:])

            nc.scalar.dma_start(ht_sb[:, 0, 0:4], hta[:])
            nc.scalar.dma_start(ht_sb[:, 0, 4:8], htb[:])
            nc.scalar.dma_start(u_sb[:, 1:2], u1[:])
            nc.scalar.dma_start(wb_sb[:], wb[:])

            # ---- epilogue helper: PSUM -> SBUF with per-partition bias --
            flip = [0]

            def bias_add(dst, src, scal):
                if flip[0] % 2 == 0:
                    nc.vector.tensor_scalar(dst, src, scal, None, ADD)
                else:
                    nc.scalar.add(dst, src, scal)
                flip[0] += 1

            # ---- mm1: t1t[k,i] = sum_d U[d,k] HT[d,i]  (+ w2[k]) --------
            for h in range(2):
                for kt in range(DO):
                    ps = mm_ps.tile([P, NH], F32, tag="mm")
                    for do in range(DO):
                        nc.tensor.matmul(
                            ps[:],
                            u_sb[:, kt, do, :],
                            ht_sb[:, h, do, :],
                            start=(do == 0), stop=(do == DO - 1),
                        )
                    bias_add(
                        t1t_sb[:, kt, h * NH:(h + 1) * NH], ps[:],
                        wb_sb[:, kt:kt + 1],
                    )
                    if h == 0 and kt == 1:
                        # gen ht1's descriptors only now (ACT queue reaches
                        # this after the kt1 epilogue) so its 1MB transfer
                        # doesn't compete with the critical startup chunks
                        nc.scalar.dma_start(ht_sb[:, 1], ht1[:])

            # ---- mm2: out[i,j] = sum_k t1t[k,i] PT'[k,j]  (+ b0') -------
            # j-halves paired per stationary t1t block; one merged [P,1024]
            # out DMA per row-block
            for it in range(SO):
                psA = mm_ps.tile([P, NH], F32, tag="mm")
                psB = mm_ps.tile([P, NH], F32, tag="mm")
                for kt in range(DO):
                    lhsT = t1t_sb[:, kt, it * P:(it + 1) * P]
                    nc.tensor.matmul(
                        psA[:], lhsT, pt_sb[:, 0, kt, :],
                        start=(kt == 0), stop=(kt == DO - 1),
                    )
                    nc.tensor.matmul(
                        psB[:], lhsT, pt_sb[:, 1, kt, :],
                        start=(kt == 0), stop=(kt == DO - 1),
                    )
                ot = outp.tile([P, 2 * NH], F32, tag="out")
                rows = slice(it * P, (it + 1) * P)
                b_ap = wb_sb[:, DO:DO + 1]
                if it < SO - 1:
                    bias_add(ot[:, 0:NH], psA[:], b_ap)
                    bias_add(ot[:, NH:2 * NH], psB[:], b_ap)
                    nc.sync.dma_start(out[rows, :], ot[:])
                else:
                    # final row-block: short tail chain — the two halves
                    # drain on separate engines and separate HWDGE rings
                    nc.scalar.add(ot[:, 0:NH], psA[:], b_ap)
                    nc.scalar.dma_start(out[rows, 0:NH], ot[:, 0:NH])
                    nc.vector.tensor_scalar(
                        ot[:, NH:2 * NH], psB[:], b_ap, None, ADD)
                    nc.sync.dma_start(
                        out[rows, NH:2 * NH], ot[:, NH:2 * NH])

    nc.compile()
    return nc


def _get_nc():
    if "nc" not in _CACHE:
        _CACHE["nc"] = build_nc()
    return _CACHE["nc"]


def _in_maps(head, dep, edge_U, edge_W, edge_b):
    head = np.asarray(head, dtype=np.float32)
    dep = np.asarray(dep, dtype=np.float32)
    U = np.asarray(edge_U, dtype=np.float32)
    w = np.asarray(edge_W, dtype=np.float32).reshape(-1)
    w1, w2 = w[:D].astype(np.float64), w[D:].astype(np.float64)

    c64 = np.linalg.solve(U.astype(np.float64), w1)    # U c = w1
    b0p = float(np.asarray(edge_b, np.float64).reshape(-1)[0]) - float(w2 @ c64)
    c = c64.astype(np.float32)

    # [kt, dd, do, k5] -> [dd, kt, do, k5]
    u_prep = np.ascontiguousarray(
        U.reshape(DO, P, DO, P).transpose(1, 2, 0, 3)
    ).astype(BF)
    wb = np.zeros((P, 16), np.float32)
    wb[:, 0:DO] = w2.astype(np.float32).reshape(DO, P).T
    wb[:, DO] = b0p

    def chunk(a, sl):
        return np.ascontiguousarray(a[:, sl])

    maps = []
    for b in range(B):
        HT = head[b].T                                  # [d, i]
        # [do, dd, h, i5] -> [dd, h, do, i5]
        ht_prep = np.ascontiguousarray(
            HT.reshape(DO, P, 2, NH).transpose(1, 2, 0, 3)
        ).astype(BF)
        PTp = dep[b].T + c[:, None]                     # [k, j]
        # [kt, kk, jh, j5] -> [kk, jh, kt, j5]
        pt_prep = np.ascontiguousarray(
            PTp.reshape(DO, P, 2, NH).transpose(1, 2, 0, 3)
        ).astype(BF)
        maps.append({
            "u0": chunk(u_prep, slice(0, 1)),
            "u1": chunk(u_prep, slice(1, 2)),
            "u23": chunk(u_prep, slice(2, 4)),
            "u47": chunk(u_prep, slice(4, 8)),
            "hta": np.ascontiguousarray(ht_prep[:, 0, 0:4]),
            "htb": np.ascontiguousarray(ht_prep[:, 0, 4:8]),
            "ht1": np.ascontiguousarray(ht_prep[:, 1]),
            "pt0": np.ascontiguousarray(pt_prep[:, 0]),
            "pt1": np.ascontiguousarray(pt_prep[:, 1]),
            "wb": wb,
        })
    return maps


def kernel(head, dep, edge_U, edge_W, edge_b, **run_kwargs):
    nc = _get_nc()
    maps = _in_maps(head, dep, edge_U, edge_W, edge_b)
    res = run_bass_kernel_spmd(nc, maps, core_ids=list(range(B)), **run_kwargs)
    out = np.stack([res.results[c]["out"] for c in range(B)], axis=0)
    if run_kwargs:
        _CACHE["last_result"] = res
    return out


# revision 19
# speedup vs baseline: 1.1016x; 1.1016x over previous
"""Biaffine edge attention on 8 Trainium2 NeuronCores.

out[b,i,j] = head[b,i,:] @ U @ dep[b,j,:] + head[b,i,:]@w1 + dep[b,j,:]@w2 + b0

Sharding: data-parallel over batch (B=8, one batch per core).

Formulation (all relayout / algebraic folding done host-side):
  HT = head[b].T                               (host relayout)
  T1T[k,i] = sum_d U[d,k] * HT[d,i]            (mm1, device)
  t1t[k,i] = T1T[k,i] + w2[k]                  (fold into the PSUM->SBUF
                                                copy: yields +s_dep[j] after
                                                mm2, since s_dep[j] =
                                                sum_k w2[k] PT[k,j])
  PT'[k,j] = dep[b].T[k,j] + c[k],  U c = w1   (host solve + relayout:
                                                yields +s_head[i] after mm2,
                                                since sum_k T1T[k,i] c[k] =
                                                head_i @ (U c) = s_head[i])
  out[i,j] = sum_k t1t[k,i] * PT'[k,j] + b0'   b0' = b0 - w2 . c

Device work = the 2 * S*D*D MAC roofline (256 N=512 matmuls) plus 32
bias-add PSUM evacuations (alternating DVE/ACT). No on-device transposes.
Inputs stream as bf16 (rel err ~5e-3 vs the 2e-2 gate), PSUM accumulates
fp32.

Schedule notes (from traces): HWDGE descriptor-gen costs ~0.7us per
dma_start regardless of size, per-ring transfers are FIFO, and the two
HWDGE rings (sync + scalar) share the SDMA engines. Every input chunk is
its own fully-contiguous dram tensor (dense HBM bursts); the two chunks
needed first (u kt=0 and ht h0/do0-3) gen in parallel on the two rings.
ht h1 descriptors are generated mid-stream from the ACT queue so the 1MB
transfer does not compete with the critical startup chunks. A burst of
matmuls on a zeroed tile keeps the PE clock warm (HAM) until the first
real data lands; mm2 merges each row-block into one [P,1024] out DMA.
"""

import numpy as np
import ml_dtypes

import concourse.bass as bass
import concourse.mybir as mybir
import concourse.tile as tile
from concourse import bacc
from concourse.bass_utils import run_bass_kernel_spmd

B, S, D = 8, 1024, 1024
P = 128
DO = D // P   # 8
SO = S // P   # 8
NH = 512      # matmul free-dim tile (one fp32 PSUM bank)
F32 = mybir.dt.float32
BF16 = mybir.dt.bfloat16
ADD = mybir.AluOpType.add
BF = ml_dtypes.bfloat16

N_DUMMY = 12  # HAM warm-up matmuls on zeroed data during initial DMA fill

_CACHE = {}


def build_nc():
    nc = bacc.Bacc(None, target_bir_lowering=False)

    # one dram tensor per DMA chunk, each contiguous, layout == sbuf tile
    # slice (partition dim first):
    #   ht*[dd, do, i5] = head.T[do*P+dd, i]     (h0 split in two, h1 whole)
    #   pt*[kk, kt, j5] = (dep.T + c)[kt*P+kk, j]
    #   u*[dd, kt, do, k5] = U[do*P+dd, kt*P+k5]
    u0 = nc.dram_tensor("u0", [P, 1, DO, P], BF16, kind="ExternalInput")
    u1 = nc.dram_tensor("u1", [P, 1, DO, P], BF16, kind="ExternalInput")
    u23 = nc.dram_tensor("u23", [P, 2, DO, P], BF16, kind="ExternalInput")
    u47 = nc.dram_tensor("u47", [P, 4, DO, P], BF16, kind="ExternalInput")
    hta = nc.dram_tensor("hta", [P, 4, NH], BF16, kind="ExternalInput")
    htb = nc.dram_tensor("htb", [P, 4, NH], BF16, kind="ExternalInput")
    ht1 = nc.dram_tensor("ht1", [P, DO, NH], BF16, kind="ExternalInput")
    pt0 = nc.dram_tensor("pt0", [P, DO, NH], BF16, kind="ExternalInput")
    pt1 = nc.dram_tensor("pt1", [P, DO, NH], BF16, kind="ExternalInput")
    wb = nc.dram_tensor("wb", [P, 16], F32, kind="ExternalInput")
    out = nc.dram_tensor("out", [S, S], F32, kind="ExternalOutput")

    with tile.TileContext(nc) as tc:
        with (
            tc.tile_pool(name="const", bufs=1) as const,
            tc.tile_pool(name="big", bufs=1) as big,
            tc.tile_pool(name="outp", bufs=3) as outp,
            tc.tile_pool(name="mm_ps", bufs=8, space="PSUM") as mm_ps,
        ):
            wb_sb = const.tile([P, 16], F32)
            dummy = const.tile([P, NH], BF16)

            u_sb = big.tile([P, DO, DO, P], BF16, tag="u")     # [dd, kt, do, k]
            ht_sb = big.tile([P, 2, DO, NH], BF16, tag="ht")   # [dd, h, do, i]
            pt_sb = big.tile([P, 2, DO, NH], BF16, tag="pt")   # [kk, jh, kt, j]
            t1t_sb = big.tile([P, DO, S], BF16, tag="t1t")     # [kk, kt, i]

            # ---- PE warm-up on zeroed data (no DMA dependency) ----------
            nc.gpsimd.memset(dummy[:], 0)
            dps = mm_ps.tile([P, NH], F32, tag="mm")
            for i in range(N_DUMMY):
                nc.tensor.matmul(
                    dps[:], dummy[:, 0:P], dummy[:],
                    start=(i == 0), stop=(i == N_DUMMY - 1),
                )

            # ---- input DMAs (gen order = per-ring FIFO priority) --------
            nc.sync.dma_start(u_sb[:, 0:1], u0[:])
            nc.sync.dma_start(ht_sb[:, 0, 4:8], htb[:])
            nc.sync.dma_start(u_sb[:, 2:4], u23[:])
            nc.sync.dma_start(u_sb[:, 4:8], u47[:])
            nc.sync.dma_start(pt_sb[:, 0], pt0[:])
            nc.sync.dma_start(pt_sb[:, 1], pt1[:])

            nc.scalar.dma_start(ht_sb[:, 0, 0:4], hta[:])
            nc.scalar.dma_start(u_sb[:, 1:2], u1[:])
            nc.scalar.dma_start(wb_sb[:], wb[:])

            # ---- epilogue helper: PSUM -> SBUF with per-partition bias --
            flip = [0]

            def bias_add(dst, src, scal):
                if flip[0] % 2 == 0:
                    nc.vector.tensor_scalar(dst, src, scal, None, ADD)
                else:
                    nc.scalar.add(dst, src, scal)
                flip[0] += 1

            # ---- mm1: t1t[k,i] = sum_d U[d,k] HT[d,i]  (+ w2[k]) --------
            for h in range(2):
                for kt in range(DO):
                    ps = mm_ps.tile([P, NH], F32, tag="mm")
                    for do in range(DO):
                        nc.tensor.matmul(
                            ps[:],
                            u_sb[:, kt, do, :],
                            ht_sb[:, h, do, :],
                            start=(do == 0), stop=(do == DO - 1),
                        )
                    bias_add(
                        t1t_sb[:, kt, h * NH:(h + 1) * NH], ps[:],
                        wb_sb[:, kt:kt + 1],
                    )
                    if h == 0 and kt == 1:
                        # gen ht1's descriptors only now (ACT queue reaches
                        # this after the kt1 epilogue) so its 1MB transfer
                        # doesn't compete with the critical startup chunks
                        nc.scalar.dma_start(ht_sb[:, 1], ht1[:])

            # ---- mm2: out[i,j] = sum_k t1t[k,i] PT'[k,j]  (+ b0') -------
            # j-halves paired per stationary t1t block; one merged [P,1024]
            # out DMA per row-block
            for it in range(SO):
                psA = mm_ps.tile([P, NH], F32, tag="mm")
                psB = mm_ps.tile([P, NH], F32, tag="mm")
                for kt in range(DO):
                    lhsT = t1t_sb[:, kt, it * P:(it + 1) * P]
                    nc.tensor.matmul(
                        psA[:], lhsT, pt_sb[:, 0, kt, :],
                        start=(kt == 0), stop=(kt == DO - 1),
                    )
                    nc.tensor.matmul(
                        psB[:], lhsT, pt_sb[:, 1, kt, :],
                        start=(kt == 0), stop=(kt == DO - 1),
                    )
                ot = outp.tile([P, 2 * NH], F32, tag="out")
                rows = slice(it * P, (it + 1) * P)
                b_ap = wb_sb[:, DO:DO + 1]
                if it < SO - 1:
                    bias_add(ot[:, 0:NH], psA[:], b_ap)
                    bias_add(ot[:, NH:2 * NH], psB[:], b_ap)
                    nc.sync.dma_start(out[rows, :], ot[:])
                else:
                    # final row-block: short tail chain — the two halves
                    # drain on separate engines and separate HWDGE rings
                    nc.scalar.add(ot[:, 0:NH], psA[:], b_ap)
                    nc.scalar.dma_start(out[rows, 0:NH], ot[:, 0:NH])
                    nc.vector.tensor_scalar(
                        ot[:, NH:2 * NH], psB[:], b_ap, None, ADD)
                    nc.sync.dma_start(
                        out[rows, NH:2 * NH], ot[:, NH:2 * NH])

    nc.compile()
    return nc


def _get_nc():
    if "nc" not in _CACHE:
        _CACHE["nc"] = build_nc()
    return _CACHE["nc"]


def _in_maps(head, dep, edge_U, edge_W, edge_b):
    head = np.asarray(head, dtype=np.float32)
    dep = np.asarray(dep, dtype=np.float32)
    U = np.asarray(edge_U, dtype=np.float32)
    w = np.asarray(edge_W, dtype=np.float32).reshape(-1)
    w1, w2 = w[:D].astype(np.float64), w[D:].astype(np.float64)

    c64 = np.linalg.solve(U.astype(np.float64), w1)    # U c = w1
    b0p = float(np.asarray(edge_b, np.float64).reshape(-1)[0]) - float(w2 @ c64)
    c = c64.astype(np.float32)

    # [kt, dd, do, k5] -> [dd, kt, do, k5]
    u_prep = np.ascontiguousarray(
        U.reshape(DO, P, DO, P).transpose(1, 2, 0, 3)
    ).astype(BF)
    wb = np.zeros((P, 16), np.float32)
    wb[:, 0:DO] = w2.astype(np.float32).reshape(DO, P).T
    wb[:, DO] = b0p

    def chunk(a, sl):
        return np.ascontiguousarray(a[:, sl])

    maps = []
    for b in range(B):
        HT = head[b].T                                  # [d, i]
        # [do, dd, h, i5] -> [dd, h, do, i5]
        ht_prep = np.ascontiguousarray(
            HT.reshape(DO, P, 2, NH).transpose(1, 2, 0, 3)
        ).astype(BF)
        PTp = dep[b].T + c[:, None]                     # [k, j]
        # [kt, kk, jh, j5] -> [kk, jh, kt, j5]
        pt_prep = np.ascontiguousarray(
            PTp.reshape(DO, P, 2, NH).transpose(1, 2, 0, 3)
        ).astype(BF)
        maps.append({
            "u0": chunk(u_prep, slice(0, 1)),
            "u1": chunk(u_prep, slice(1, 2)),
            "u23": chunk(u_prep, slice(2, 4)),
            "u47": chunk(u_prep, slice(4, 8)),
            "hta": np.ascontiguousarray(ht_prep[:, 0, 0:4]),
            "htb": np.ascontiguousarray(ht_prep[:, 0, 4:8]),
            "ht1": np.ascontiguousarray(ht_prep[:, 1]),
            "pt0": np.ascontiguousarray(pt_prep[:, 0]),
            "pt1": np.ascontiguousarray(pt_prep[:, 1]),
            "wb": wb,
        })
    return maps


def kernel(head, dep, edge_U, edge_W, edge_b, **run_kwargs):
    nc = _get_nc()
    maps = _in_maps(head, dep, edge_U, edge_W, edge_b)
    res = run_bass_kernel_spmd(nc, maps, core_ids=list(range(B)), **run_kwargs)
    out = np.stack([res.results[c]["out"] for c in range(B)], axis=0)
    if run_kwargs:
        _CACHE["last_result"] = res
    return out
